# revision 1
# baseline (speedup 1.0000x reference)
"""KAN (Kolmogorov-Arnold Network) forward kernel for Trainium2, 8 NeuronCores.

Network: WIDTH=[64,128,64], BATCH=2048, cubic B-splines (K=3) on a shared
uniform grid of G=5 intervals over [-1,1], plus a SiLU residual branch per
edge.

Math: with a uniform shared knot vector, every edge's spline is a linear
combination of *shared* basis functions of its input scalar:
    spline_s(x) = sum_c coef[s,c] * M3(u - c),   u = (x - e0)/h
where M3 is the cardinal cubic B-spline (support [0,4]) and e0/h describe the
extended knot grid.  M3 decomposes into two bounded "tent-cube" functions
    M3(t) = ( relu(min(t, 4-t))^3 - 4*relu(min(t-1, 3-t))^3 ) / 6
so a whole KAN layer becomes:
  1. elementwise tent-cube features of the input scalars  (one custom DVE op)
  2. a single dense matmul with host-repacked coefficients (TensorE, fp16)
  3. the SiLU branch folds in as one extra feature block.
Features are bounded (<= 8), so fp16 is numerically safe (no cancellation),
validated at ~5e-4 rel error vs the fp32 reference.

Sharding: data-parallel over batch (2048/8 = 256 per core); spline params are
repacked on host and replicated to all 8 cores.  No collectives.
"""

import os
import numpy as np

# ---------------------------------------------------------------- constants
_BATCH = 2048
_W = [64, 128, 64]
_NCORES = 8
_NB = _BATCH // _NCORES          # batch per core = 256
_K = 3                           # spline order
_G = 5                           # grid intervals
_NBASIS = _G + _K                # 8 basis functions per edge

LAST_RESULTS = None              # BassKernelResults of the most recent run


# ------------------------------------------------------------ numpy fallback
def _np_reference(inputs):
    """Exact numpy port of the reference (used only if the structural
    assumptions about the inputs do not hold)."""
    def extend_grid(grid, k):
        h = (grid[:, -1:] - grid[:, :1]) / (grid.shape[1] - 1)
        for _ in range(k):
            grid = np.concatenate([grid[:, :1] - h, grid, grid[:, -1:] + h], axis=1)
        return grid

    def b_spline_basis(x, grid, k):
        g = grid[:, :, None]
        xe = x[:, None, :]
        val = ((xe >= g[:, :-1]) & (xe < g[:, 1:])).astype(x.dtype)
        for p in range(1, k + 1):
            left = (xe - g[:, :-(p + 1)]) / (g[:, p:-1] - g[:, :-(p + 1)])
            right = (g[:, p + 1:] - xe) / (g[:, p + 1:] - xe * 0 - g[:, 1:-p])
            val = left * val[:, :-1] + right * val[:, 1:]
        return val

    def silu(v):
        return v / (1.0 + np.exp(-v))

    def kan_layer(x, grid, coef, scale_base, scale_sp, mask, in_dim, out_dim):
        batch = x.shape[0]
        xe = np.broadcast_to(x[:, None, :], (batch, out_dim, in_dim)).reshape(batch, -1).T
        base = silu(xe)
        B = b_spline_basis(xe, extend_grid(grid, _K), _K)
        spline = np.einsum('sc,scb->sb', coef, B)
        y = mask[:, None] * (scale_base[:, None] * base + scale_sp[:, None] * spline)
        return y.T.reshape(batch, out_dim, in_dim).sum(axis=2)

    x = np.asarray(inputs["x"], np.float32)
    h = kan_layer(x, inputs["grid0"], inputs["coef0"], inputs["scale_base0"],
                  inputs["scale_sp0"], inputs["mask0"], _W[0], _W[1]) + inputs["bias0"][None, :]
    out = kan_layer(h.astype(np.float32), inputs["grid1"], inputs["coef1"], inputs["scale_base1"],
                    inputs["scale_sp1"], inputs["mask1"], _W[1], _W[2]) + inputs["bias1"][None, :]
    return out.astype(np.float32)


def _grid_params(grid):
    """(e0, h) of the extended uniform grid; replicates extend_grid fp32 steps."""
    g0 = np.asarray(grid, np.float32)[0]
    h = np.float32((g0[-1] - g0[0]) / _G)
    e0 = np.float32(g0[0])
    for _ in range(_K):
        e0 = np.float32(e0 - h)
    return float(e0), float(h)


def _uniform_shared(grid):
    g = np.asarray(grid, np.float32)
    if not np.allclose(g, g[0][None, :], atol=1e-6):
        return False
    g0 = g[0]
    d = np.diff(g0)
    return np.allclose(d, d[0], rtol=1e-4, atol=1e-6) and d[0] > 0


# ------------------------------------------------------- custom DVE op setup
def _register_tent_cube():
    """Register the KAN_TENT_CUBE custom DVE op:
        out = relu(min(d, s1 - d))^3,  d = in0 - in1
    in0 = u values (broadcast over the feature axis), in1 = per-feature knot
    offsets, s1 = tent width (4.0 for the A family, 2.0 for the B family)."""
    import dataclasses
    from concourse import dve_ops
    from concourse.dve_spec import Spec, Src0, Src1, C1, relu, sq, minn, lower, _has_src1
    from concourse.dve_uop import DveOpSpec

    name = "KAN_TENT_CUBE"
    for op in dve_ops.OPS:
        if op.name == name:
            return op

    d = Src0 - Src1
    r = relu(minn(d, C1 - d))

    def _ref(in0, in1, s0, s1, imm2):
        dd = in0.astype(np.float32) - in1.astype(np.float32)
        rr = np.maximum(np.minimum(dd, np.float32(s1) - dd), 0.0)
        return (rr * rr * rr).astype(np.float32)

    spec = Spec(body=sq(r) * r, reference=_ref)
    probe = dataclasses.replace(
        dve_ops.DveOp(name, spec, subdim=False, uops_sha={}), uops_sha={})
    dve_ops.OPS.append(probe)
    row = dve_ops._CUSTOM_DVE_ROW_BASE + len(dve_ops.OPS) - 1
    dve_ops._SUB_OPCODE_FOR_NAME[name] = row
    dve_ops.CUSTOM_DVE_SPECS[name] = spec

    shas = {}
    for ver in ("v3", "v4"):
        try:
            shas[ver] = DveOpSpec(
                name=name, opcode=row, uops=lower(spec, ver=ver),
                rd1_en=_has_src1(spec)).sha(ver)
        except Exception:
            pass
    final = dataclasses.replace(probe, uops_sha=shas)
    dve_ops.OPS[dve_ops.OPS.index(probe)] = final
    return final


# ------------------------------------------------------------- host repacking
def _pack_layer0(coef, scale_sp, scale_base, mask):
    """W0 pack: (128, 9, 128) fp16.  Chunk rows p=(j,i): j=p//64 in {0,1},
    i=p%64 input channel.  Chunks 0..3 = A features n=2s+j, 4..7 = B features
    (weight -4x), 8 = silu rows (i<64) + zero pad."""
    C = (mask * scale_sp)[:, None] * coef / 6.0                # (8192, 8)
    C = C.reshape(_W[1], _W[0], _NBASIS)                       # (o, i, c)
    WB = (mask * scale_base).reshape(_W[1], _W[0])             # (o, i)
    w = np.zeros((128, 9, 128), np.float32)
    for s in range(4):
        for j in range(2):
            w[j * 64:(j + 1) * 64, s, :] = C[:, :, 2 * s + j].T
    w[:, 4:8, :] = -4.0 * w[:, 0:4, :]
    w[0:64, 8, :] = WB.T
    return w.astype(np.float16)


def _pack_layer1(coef, scale_sp, scale_base, mask):
    """W1 pack: (128, 17, 64) fp16.  Rows p = input channel (128).  Chunks
    0..7 = A_s, 8..15 = B_s (weight -4x), 16 = silu."""
    C = (mask * scale_sp)[:, None] * coef / 6.0
    C = C.reshape(_W[2], _W[1], _NBASIS)                       # (o, i, c)
    WB = (mask * scale_base).reshape(_W[2], _W[1])
    w = np.zeros((128, 17, 64), np.float32)
    for s in range(_NBASIS):
        w[:, s, :] = C[:, :, s].T
        w[:, 8 + s, :] = -4.0 * C[:, :, s].T
    w[:, 16, :] = WB.T
    return w.astype(np.float16)


def _make_tabs(e0_0, e0_1):
    """(128, 24) fp32 per-partition knot-offset tables.
    cols 0:4  = layer0 A offsets  n = 2k + (p//64)
    cols 4:8  = layer0 B offsets  n+1
    cols 8:16 = layer1 A offsets  s
    cols 16:24= layer1 B offsets  s+1
    col 24 = e0_0, col 25 = e0_1 (activation bias vectors)"""
    t = np.zeros((128, 26), np.float32)
    p = np.arange(128)
    j = (p // 64).astype(np.float32)
    for k in range(4):
        t[:, k] = 2 * k + j
        t[:, 4 + k] = 2 * k + j + 1.0
    for s in range(8):
        t[:, 8 + s] = s
        t[:, 16 + s] = s + 1.0
    t[:, 24] = e0_0
    t[:, 25] = e0_1
    return t


# ------------------------------------------------------------- bass program
def _build_program(e0_0, h0, e0_1, h1):
    import concourse.bacc as bacc
    import concourse.tile as tile
    import concourse.mybir as mybir

    _register_tent_cube()
    from concourse import dve_ops
    OP = next(o for o in dve_ops.OPS if o.name == "KAN_TENT_CUBE")
    AMR = dve_ops.AFFINE_MUL_REDUCE

    dt = mybir.dt
    AF = mybir.ActivationFunctionType

    nc = bacc.Bacc("TRN2", target_bir_lowering=False, debug=False,
                   num_devices=_NCORES)

    xu_d = nc.dram_tensor("xu", [64, _NB], dt.float32, kind="ExternalInput").ap()
    w0_d = nc.dram_tensor("w0", [128, 9 * 128], dt.float16, kind="ExternalInput").ap()
    w1_d = nc.dram_tensor("w1", [128, 17 * 64], dt.float16, kind="ExternalInput").ap()
    tabs_d = nc.dram_tensor("tabs", [128, 26], dt.float32, kind="ExternalInput").ap()
    bv0_d = nc.dram_tensor("bv0", [128, 1], dt.float32, kind="ExternalInput").ap()
    bv1_d = nc.dram_tensor("bv1", [64, 1], dt.float32, kind="ExternalInput").ap()
    out_d = nc.dram_tensor("out", [64, _NB], dt.float32, kind="ExternalOutput").ap()

    import contextlib
    with tile.TileContext(nc) as tc, contextlib.ExitStack() as _ctx:
        pool = _ctx.enter_context(tc.tile_pool(name="main", bufs=1))
        ppool = _ctx.enter_context(tc.tile_pool(name="ps", bufs=1, space="PSUM"))
        u2 = pool.tile([128, _NB], dt.float32, name="u2", tag="u2")
        w0sb = pool.tile([128, 9 * 128], dt.float16, name="w0sb", tag="w0sb")
        w1sb = pool.tile([128, 17 * 64], dt.float16, name="w1sb", tag="w1sb")
        tabs = pool.tile([128, 26], dt.float32, name="tabs", tag="tabs")
        bv0 = pool.tile([128, 1], dt.float32, name="bv0", tag="bv0")
        bv1 = pool.tile([64, 1], dt.float32, name="bv1", tag="bv1")
        fa0 = pool.tile([128, 4 * _NB], dt.float16, name="fa0", tag="fa0")
        fb0 = pool.tile([128, 4 * _NB], dt.float16, name="fb0", tag="fb0")
        s0t = pool.tile([128, _NB], dt.float16, name="s0t", tag="s0t")
        sg0 = pool.tile([64, _NB], dt.float32, name="sg0", tag="sg0")
        sg1 = pool.tile([128, _NB], dt.float32, name="sg1", tag="sg1")
        u1 = pool.tile([128, _NB], dt.float32, name="u1", tag="u1")
        fa1 = pool.tile([128, 8 * _NB], dt.float16, name="fa1", tag="fa1")
        fb1 = pool.tile([128, 8 * _NB], dt.float16, name="fb1", tag="fb1")
        s1t = pool.tile([128, _NB], dt.float16, name="s1t", tag="s1t")
        outT = pool.tile([64, _NB], dt.float32, name="outT", tag="outT")
        ph = ppool.tile([128, _NB], dt.float32, name="ph", tag="ph")
        po = ppool.tile([64, _NB], dt.float32, name="po", tag="po")

        # loads
        nc.sync.dma_start(out=u2[0:64, :], in_=xu_d)
        nc.sync.dma_start(out=u2[64:128, :], in_=xu_d)
        nc.sync.dma_start(out=tabs[:, :], in_=tabs_d)
        nc.sync.dma_start(out=w0sb[:, :], in_=w0_d)
        nc.sync.dma_start(out=w1sb[:, :], in_=w1_d)
        nc.sync.dma_start(out=bv0[:, :], in_=bv0_d)
        nc.sync.dma_start(out=bv1[:, :], in_=bv1_d)

        def bcast_in0(src, S):
            return src.rearrange("p (s n) -> p s n", s=1).broadcast_to([128, S, _NB])

        def bcast_tab(sl, S):
            return sl.rearrange("p (s n) -> p s n", n=1).broadcast_to([128, S, _NB])

        # ---- layer 0 features
        for hh in range(2):
            sl = slice(hh * 2 * _NB, (hh + 1) * 2 * _NB)
            nc.vector._custom_dve(
                OP, out=fa0[:, sl].rearrange("p (s n) -> p s n", s=2),
                in0=bcast_in0(u2[:, :], 2),
                in1=bcast_tab(tabs[:, 2 * hh:2 * hh + 2], 2), s1=4.0)
        for hh in range(2):
            sl = slice(hh * 2 * _NB, (hh + 1) * 2 * _NB)
            nc.vector._custom_dve(
                OP, out=fb0[:, sl].rearrange("p (s n) -> p s n", s=2),
                in0=bcast_in0(u2[:, :], 2),
                in1=bcast_tab(tabs[:, 4 + 2 * hh:6 + 2 * hh], 2), s1=2.0)
        nc.gpsimd.memset(s0t[64:128, :], 0.0)
        nc.scalar.activation(sg0[:, :], u2[0:64, :], AF.Sigmoid,
                             bias=tabs[0:64, 24:25], scale=float(h0))
        nc.vector._custom_dve(AMR, out=s0t[0:64, :], in0=u2[0:64, :],
                              in1=sg0[:, :], s0=float(h0), s1=float(e0_0))

        # ---- layer 0 matmuls: h^T = W0^T @ F0   (psum accumulate)
        n_mm0 = 9
        idx = 0
        for s in range(4):
            nc.tensor.matmul(ph[:, :], lhsT=w0sb[:, s * 128:(s + 1) * 128],
                             rhs=fa0[:, s * _NB:(s + 1) * _NB],
                             start=(idx == 0), stop=(idx == n_mm0 - 1))
            idx += 1
        for s in range(4):
            nc.tensor.matmul(ph[:, :], lhsT=w0sb[:, (4 + s) * 128:(5 + s) * 128],
                             rhs=fb0[:, s * _NB:(s + 1) * _NB],
                             start=(idx == 0), stop=(idx == n_mm0 - 1))
            idx += 1
        nc.tensor.matmul(ph[:, :], lhsT=w0sb[:, 8 * 128:9 * 128], rhs=s0t[:, :],
                         start=(idx == 0), stop=(idx == n_mm0 - 1))

        # ---- evacuate h -> u1 = (h + bias0 - e0_1)/h1  (bias folded on host)
        nc.scalar.activation(u1[:, :], ph[:, :], AF.Identity,
                             bias=bv0[:, :], scale=float(1.0 / h1))

        # ---- layer 1 features
        for hh in range(2):
            sl = slice(hh * 4 * _NB, (hh + 1) * 4 * _NB)
            nc.vector._custom_dve(
                OP, out=fa1[:, sl].rearrange("p (s n) -> p s n", s=4),
                in0=bcast_in0(u1[:, :], 4),
                in1=bcast_tab(tabs[:, 8 + 4 * hh:12 + 4 * hh], 4), s1=4.0)
        for hh in range(2):
            sl = slice(hh * 4 * _NB, (hh + 1) * 4 * _NB)
            nc.vector._custom_dve(
                OP, out=fb1[:, sl].rearrange("p (s n) -> p s n", s=4),
                in0=bcast_in0(u1[:, :], 4),
                in1=bcast_tab(tabs[:, 16 + 4 * hh:20 + 4 * hh], 4), s1=2.0)
        nc.scalar.activation(sg1[:, :], u1[:, :], AF.Sigmoid,
                             bias=tabs[:, 25:26], scale=float(h1))
        nc.vector._custom_dve(AMR, out=s1t[:, :], in0=u1[:, :],
                              in1=sg1[:, :], s0=float(h1), s1=float(e0_1))

        # ---- layer 1 matmuls
        n_mm1 = 17
        idx = 0
        for s in range(8):
            nc.tensor.matmul(po[:, :], lhsT=w1sb[:, s * 64:(s + 1) * 64],
                             rhs=fa1[:, s * _NB:(s + 1) * _NB],
                             start=(idx == 0), stop=(idx == n_mm1 - 1))
            idx += 1
        for s in range(8):
            nc.tensor.matmul(po[:, :], lhsT=w1sb[:, (8 + s) * 64:(9 + s) * 64],
                             rhs=fb1[:, s * _NB:(s + 1) * _NB],
                             start=(idx == 0), stop=(idx == n_mm1 - 1))
            idx += 1
        nc.tensor.matmul(po[:, :], lhsT=w1sb[:, 16 * 64:17 * 64], rhs=s1t[:, :],
                         start=(idx == 0), stop=(idx == n_mm1 - 1))

        # ---- out^T = psum + bias1
        nc.scalar.activation(outT[:, :], po[:, :], AF.Identity,
                             bias=bv1[:, :], scale=1.0)
        nc.sync.dma_start(out=out_d, in_=outT[:, :])

    nc.compile()
    return nc


def _make_in_maps(inputs, e0_0, h0, e0_1, h1):
    x = np.asarray(inputs["x"], np.float32)
    u_full = ((x - np.float32(e0_0)) / np.float32(h0)).astype(np.float32)

    w0 = _pack_layer0(np.asarray(inputs["coef0"], np.float32),
                      np.asarray(inputs["scale_sp0"], np.float32),
                      np.asarray(inputs["scale_base0"], np.float32),
                      np.asarray(inputs["mask0"], np.float32)).reshape(128, 9 * 128)
    w1 = _pack_layer1(np.asarray(inputs["coef1"], np.float32),
                      np.asarray(inputs["scale_sp1"], np.float32),
                      np.asarray(inputs["scale_base1"], np.float32),
                      np.asarray(inputs["mask1"], np.float32)).reshape(128, 17 * 64)
    tabs = _make_tabs(e0_0, e0_1)
    bias0 = np.asarray(inputs["bias0"], np.float32)
    bias1 = np.asarray(inputs["bias1"], np.float32)
    bv0 = ((bias0 - np.float32(e0_1)) / np.float32(h1)).reshape(128, 1).astype(np.float32)
    bv1 = bias1.reshape(64, 1).astype(np.float32)

    in_maps = []
    for c in range(_NCORES):
        xu = np.ascontiguousarray(u_full[c * _NB:(c + 1) * _NB, :].T)
        in_maps.append({
            "xu": xu, "w0": np.ascontiguousarray(w0),
            "w1": np.ascontiguousarray(w1), "tabs": tabs,
            "bv0": bv0, "bv1": bv1,
        })
    return in_maps


# ---------------------------------------------------------------- entrypoint
def kernel(**inputs):
    global LAST_RESULTS
    x = np.asarray(inputs["x"])
    ok = (
        x.shape == (_BATCH, _W[0])
        and np.asarray(inputs["coef0"]).shape == (_W[0] * _W[1], _NBASIS)
        and np.asarray(inputs["coef1"]).shape == (_W[1] * _W[2], _NBASIS)
        and _uniform_shared(inputs["grid0"])
        and _uniform_shared(inputs["grid1"])
    )
    if not ok:
        return _np_reference(inputs)

    e0_0, h0 = _grid_params(inputs["grid0"])
    e0_1, h1 = _grid_params(inputs["grid1"])

    from concourse.bass_utils import run_bass_kernel_spmd

    nc = _build_program(e0_0, h0, e0_1, h1)
    in_maps = _make_in_maps(inputs, e0_0, h0, e0_1, h1)
    trace = bool(int(os.environ.get("KAN_TRACE", "0")))
    try:
        res = run_bass_kernel_spmd(nc, in_maps, list(range(_NCORES)), trace=trace)
    except ModuleNotFoundError:
        # NTFF profiling hook unavailable in this container; run untraced.
        res = run_bass_kernel_spmd(nc, in_maps, list(range(_NCORES)), trace=False)
    LAST_RESULTS = res

    out = np.empty((_BATCH, _W[2]), np.float32)
    for c in range(_NCORES):
        out[c * _NB:(c + 1) * _NB, :] = res.results[c]["out"].T
    return out


if __name__ == "__main__":
    rng = np.random.default_rng(0)
    demo = {
        "x": rng.standard_normal((_BATCH, _W[0])).astype(np.float32),
    }
    for l, size in ((0, _W[0] * _W[1]), (1, _W[1] * _W[2])):
        demo[f"grid{l}"] = np.broadcast_to(
            np.linspace(-1, 1, _G + 1, dtype=np.float32), (size, _G + 1)).copy()
        demo[f"coef{l}"] = (rng.standard_normal((size, _NBASIS)) * 0.1).astype(np.float32)
        demo[f"scale_base{l}"] = rng.standard_normal(size).astype(np.float32) * 0.1 + 0.125
        demo[f"scale_sp{l}"] = np.ones(size, np.float32)
        demo[f"mask{l}"] = np.ones(size, np.float32)
        demo[f"bias{l}"] = np.zeros((_W[1], _W[2])[l], np.float32)
    out = kernel(**demo)
    ref = _np_reference(demo)
    err = np.abs(out - ref).max() / np.abs(ref).max()
    print("demo rel err:", err)



# revision 6
# speedup vs baseline: 1.1727x; 1.1727x over previous
"""KAN (Kolmogorov-Arnold Network) forward kernel for Trainium2, 8 NeuronCores.

Network: WIDTH=[64,128,64], BATCH=2048, cubic B-splines (K=3) on a shared
uniform grid of G=5 intervals over [-1,1], plus a SiLU residual branch per
edge.

Math: on the shared uniform extended knot grid every edge's spline is
    spline_s(x) = sum_c coef[s,c] * M3(u - c),   u = (x - e0)/h
with M3 the cardinal cubic B-spline.  With a common center m = c + 2:
    6*M3(u - c) = A(u) - 4*B(u)
    A(u) = relu(2 - |u - m|)^3 ,  B(u) = relu(1 - |u - m|)^3
Two custom DVE ops produce the fused feature directly (offsets generated
in-op with a paged counter, so no offset tables are streamed):
    KAN_TENT3 :  fa[p,s,n]  = relu(C2 - |u - (C0 + s*C1)|)^3        (= A)
    KAN_M3SUB :  m[p,s,n]   = fa - (2b)^2*b,  b = relu(1 - |u - (C0+s*C1)|)
so m = A - 4B = 6*M3 and one matmul chunk per basis index contracts the
fused features (half the matmul chunks and weight bytes of an unfused A/B
split).  The SiLU residual is a native Activation-engine Silu per layer.

Sharding: data-parallel over batch (2048/8 = 256 per core); spline params
repacked on host and replicated to all 8 cores.  No collectives.
"""

import os
import numpy as np

# ---------------------------------------------------------------- constants
_BATCH = 2048
_W = [64, 128, 64]
_NCORES = 8
_NB = _BATCH // _NCORES          # batch per core = 256
_K = 3                           # spline order
_G = 5                           # grid intervals
_NBASIS = _G + _K                # 8 basis functions per edge

_N_WARM = int(os.environ.get("KAN_NWARM", "0"))   # PE warm-up matmuls
# L1 fused-feature tail split (pages per KAN_M3SUB call)
_L1_SPLIT = (4, 2, 1, 1)
_L0_SPLIT = (2, 2)

LAST_RESULTS = None              # BassKernelResults of the most recent run


# ------------------------------------------------------------ numpy fallback
def _np_reference(inputs):
    """Exact numpy port of the reference (used only if the structural
    assumptions about the inputs do not hold)."""
    def extend_grid(grid, k):
        h = (grid[:, -1:] - grid[:, :1]) / (grid.shape[1] - 1)
        for _ in range(k):
            grid = np.concatenate([grid[:, :1] - h, grid, grid[:, -1:] + h], axis=1)
        return grid

    def b_spline_basis(x, grid, k):
        g = grid[:, :, None]
        xe = x[:, None, :]
        val = ((xe >= g[:, :-1]) & (xe < g[:, 1:])).astype(x.dtype)
        for p in range(1, k + 1):
            left = (xe - g[:, :-(p + 1)]) / (g[:, p:-1] - g[:, :-(p + 1)])
            right = (g[:, p + 1:] - xe) / (g[:, p + 1:] - xe * 0 - g[:, 1:-p])
            val = left * val[:, :-1] + right * val[:, 1:]
        return val

    def silu(v):
        return v / (1.0 + np.exp(-v))

    def kan_layer(x, grid, coef, scale_base, scale_sp, mask, in_dim, out_dim):
        batch = x.shape[0]
        xe = np.broadcast_to(x[:, None, :], (batch, out_dim, in_dim)).reshape(batch, -1).T
        base = silu(xe)
        B = b_spline_basis(xe, extend_grid(grid, _K), _K)
        spline = np.einsum('sc,scb->sb', coef, B)
        y = mask[:, None] * (scale_base[:, None] * base + scale_sp[:, None] * spline)
        return y.T.reshape(batch, out_dim, in_dim).sum(axis=2)

    x = np.asarray(inputs["x"], np.float32)
    h = kan_layer(x, inputs["grid0"], inputs["coef0"], inputs["scale_base0"],
                  inputs["scale_sp0"], inputs["mask0"], _W[0], _W[1]) + inputs["bias0"][None, :]
    out = kan_layer(h.astype(np.float32), inputs["grid1"], inputs["coef1"], inputs["scale_base1"],
                    inputs["scale_sp1"], inputs["mask1"], _W[1], _W[2]) + inputs["bias1"][None, :]
    return out.astype(np.float32)


def _grid_params(grid):
    """(e0, h) of the extended uniform grid; replicates extend_grid fp32 steps."""
    g0 = np.asarray(grid, np.float32)[0]
    h = np.float32((g0[-1] - g0[0]) / _G)
    e0 = np.float32(g0[0])
    for _ in range(_K):
        e0 = np.float32(e0 - h)
    return float(e0), float(h)


def _uniform_shared(grid):
    g = np.asarray(grid, np.float32)
    if not np.allclose(g, g[0][None, :], atol=1e-6):
        return False
    g0 = g[0]
    d = np.diff(g0)
    return np.allclose(d, d[0], rtol=1e-4, atol=1e-6) and d[0] > 0


# ------------------------------------------------------- custom DVE op setup
def _register_ops():
    """Register KAN_TENT3 (A-family tent-cube) and KAN_M3SUB (fused
    m = A - 4*B) custom DVE ops.  Offsets are generated in-op by a paged
    counter: center(s) = C0 + s*C1 (C0 may be a per-partition scalar AP)."""
    import dataclasses
    from concourse import dve_ops
    from concourse.dve_spec import (Spec, Src0, Src1, C0, C1, C2, One, relu,
                                    sq, lower, _has_src1, PageIdx, Bin, AluOp)
    from concourse.dve_uop import DveOpSpec

    def reg(name, spec, subdim):
        for op in dve_ops.OPS:
            if op.name == name:
                return op
        probe = dataclasses.replace(
            dve_ops.DveOp(name, spec, subdim=subdim, uops_sha={}), uops_sha={})
        dve_ops.OPS.append(probe)
        row = dve_ops._CUSTOM_DVE_ROW_BASE + len(dve_ops.OPS) - 1
        dve_ops._SUB_OPCODE_FOR_NAME[name] = row
        dve_ops.CUSTOM_DVE_SPECS[name] = spec
        shas = {}
        for ver in ("v3", "v4"):
            try:
                shas[ver] = DveOpSpec(
                    name=name, opcode=row, uops=lower(spec, ver=ver),
                    rd1_en=_has_src1(spec)).sha(ver)
            except Exception:
                pass
        final = dataclasses.replace(probe, uops_sha=shas)
        dve_ops.OPS[dve_ops.OPS.index(probe)] = final
        return final

    pg = PageIdx(C0, C1)
    ad = Bin(AluOp.ABSOLUTE_DIFF, Src0, pg)

    ma = C2 - ad
    ra = relu(ma)

    def _ref_a(in0, in1, s0, s1, imm2):
        x = in0.astype(np.float32)
        S = x.shape[1]
        off = (np.float32(s0) if np.isscalar(s0) else s0.astype(np.float32)[:, None, None]) \
            + np.arange(S, dtype=np.float32)[None, :, None] * np.float32(s1)
        rr = np.maximum(np.float32(imm2) - np.abs(x - off), 0.0)
        return (rr ** 3).astype(np.float32)

    tent3 = reg("KAN_TENT3", Spec(body=sq(ra) * ra, reference=_ref_a), True)

    mb = One - ad
    rb = relu(mb)
    b2 = rb + rb

    def _ref_b(in0, in1, s0, s1, imm2):
        x = in0.astype(np.float32)
        S = x.shape[1]
        off = (np.float32(s0) if np.isscalar(s0) else s0.astype(np.float32)[:, None, None]) \
            + np.arange(S, dtype=np.float32)[None, :, None] * np.float32(s1)
        bb = np.maximum(1.0 - np.abs(x - off), 0.0)
        return (in1.astype(np.float32) - 4.0 * bb ** 3).astype(np.float32)

    m3sub = reg("KAN_M3SUB", Spec(body=Src1 - sq(b2) * rb, reference=_ref_b), True)
    return tent3, m3sub


# ------------------------------------------------------------- host repacking
def _pack_layer0(coef, scale_sp, scale_base, mask):
    """W0 pack: (128, 5, 128) fp16.  Feature rows p=(j,i): j=p//64, i=p%64.
    Chunks 0..3: fused 6*M3 features for basis c=2k+j -> weight C/6.
    Chunk 4: silu rows (i<64) + zero pad."""
    C = (mask * scale_sp)[:, None] * coef / 6.0                # (8192, 8)
    C = C.reshape(_W[1], _W[0], _NBASIS)                       # (o, i, c)
    WB = (mask * scale_base).reshape(_W[1], _W[0])             # (o, i)
    w = np.zeros((128, 5, 128), np.float32)
    for k in range(4):
        for j in range(2):
            w[j * 64:(j + 1) * 64, k, :] = C[:, :, 2 * k + j].T
    w[0:64, 4, :] = WB.T
    return w.astype(np.float16)


def _pack_layer1(coef, scale_sp, scale_base, mask):
    """W1 pack: (128, 9, 64) fp16.  Rows p = input channel.
    Chunks 0..7: fused 6*M3 features, chunk 8: silu."""
    C = (mask * scale_sp)[:, None] * coef / 6.0
    C = C.reshape(_W[2], _W[1], _NBASIS)                       # (o, i, c)
    WB = (mask * scale_base).reshape(_W[2], _W[1])
    w = np.zeros((128, 9, 64), np.float32)
    for s in range(_NBASIS):
        w[:, s, :] = C[:, :, s].T
    w[:, 8, :] = WB.T
    return w.astype(np.float16)


# ------------------------------------------------------------- bass program
def _build_program(e0_0, h0, e0_1, h1):
    import concourse.bacc as bacc
    import concourse.tile as tile
    import concourse.mybir as mybir

    TENT3, M3SUB = _register_ops()

    dt = mybir.dt
    AF = mybir.ActivationFunctionType

    nc = bacc.Bacc("TRN2", target_bir_lowering=False, debug=False,
                   num_devices=_NCORES)

    u16_d = nc.dram_tensor("u16", [128, _NB], dt.float16, kind="ExternalInput").ap()
    w0_d = nc.dram_tensor("w0", [128, 5 * 128], dt.float16, kind="ExternalInput").ap()
    w1_d = nc.dram_tensor("w1", [128, 9 * 64], dt.float16, kind="ExternalInput").ap()
    prm_d = nc.dram_tensor("prm", [128, 2], dt.float32, kind="ExternalInput").ap()
    out_d = nc.dram_tensor("out", [64, _NB], dt.float32, kind="ExternalOutput").ap()

    import contextlib
    with tile.TileContext(nc) as tc, contextlib.ExitStack() as _ctx:
        pool = _ctx.enter_context(tc.tile_pool(name="main", bufs=1))
        ppool = _ctx.enter_context(tc.tile_pool(name="ps", bufs=1, space="PSUM"))

        u16 = pool.tile([128, _NB], dt.float16, name="u16", tag="u16")
        w0sb = pool.tile([128, 5 * 128], dt.float16, name="w0sb", tag="w0sb")
        w1sb = pool.tile([128, 9 * 64], dt.float16, name="w1sb", tag="w1sb")
        prm = pool.tile([128, 2], dt.float32, name="prm", tag="prm")
        jc2 = pool.tile([128, 1], dt.float32, name="jc2", tag="jc2")
        jc2b = pool.tile([128, 1], dt.float32, name="jc2b", tag="jc2b")
        e00b = pool.tile([128, 1], dt.float32, name="e00b", tag="e00b")
        sb1 = pool.tile([128, 1], dt.float32, name="sb1", tag="sb1")
        wzf = pool.tile([128, 1], dt.float32, name="wzf", tag="wzf")
        wz = pool.tile([128, _NB], dt.float16, name="wz", tag="wz")
        fa0 = pool.tile([128, 4 * _NB], dt.float16, name="fa0", tag="fa0")
        m0 = pool.tile([128, 4 * _NB], dt.float16, name="m0", tag="m0")
        s0t = pool.tile([128, _NB], dt.float16, name="s0t", tag="s0t")
        u1t = pool.tile([128, _NB], dt.float16, name="u1t", tag="u1t")
        fa1 = pool.tile([128, 8 * _NB], dt.float16, name="fa1", tag="fa1")
        m1 = pool.tile([128, 8 * _NB], dt.float16, name="m1", tag="m1")
        s1t = pool.tile([128, _NB], dt.float16, name="s1t", tag="s1t")
        outT = pool.tile([64, _NB], dt.float32, name="outT", tag="outT")
        aw = pool.tile([64, 1], dt.float16, name="aw", tag="aw")
        ph = ppool.tile([128, _NB], dt.float32, name="ph", tag="ph")
        po = ppool.tile([64, _NB], dt.float32, name="po", tag="po")
        if _N_WARM:
            pw = ppool.tile([128, _NB], dt.float32, name="pw", tag="pw")

        # ---- input DMAs (SP queue; ordered by when each is first needed)
        nc.sync.dma_start(out=u16[:, :], in_=u16_d)
        nc.sync.dma_start(out=w0sb[:, :], in_=w0_d)
        nc.sync.dma_start(out=w1sb[:, :], in_=w1_d)
        nc.sync.dma_start(out=prm[:, :], in_=prm_d)

        # ---- Pool: constants (all tiny memsets, off the critical path)
        nc.gpsimd.memset(wzf[:, :], 0.0)
        nc.gpsimd.memset(wz[:, :], 0.0)
        nc.gpsimd.memset(jc2[0:64, :], 2.0)     # L0 tent centers: 2k + j + 2
        nc.gpsimd.memset(jc2[64:128, :], 3.0)
        nc.gpsimd.memset(jc2b[0:64, :], 6.0)    # second pair of L0 pages
        nc.gpsimd.memset(jc2b[64:128, :], 7.0)
        nc.gpsimd.memset(e00b[:, :], float(e0_0))
        nc.gpsimd.memset(sb1[:, :], float(e0_1))
        nc.gpsimd.memset(s0t[64:128, :], 0.0)

        # ---- Activation: preload the silu/identity table off the critical path
        if not int(os.environ.get("KAN_NOWARM_ACT", "0")):
            nc.scalar.activation(aw[:, :], wz[0:64, 0:1], AF.Silu,
                                 bias=wzf[0:64, :], scale=1.0)

        # ---- PE warm-up
        for _ in range(_N_WARM):
            nc.tensor.matmul(pw[:, :], lhsT=wz[:, 0:128], rhs=wz[:, :],
                             start=True, stop=True)

        def bcast(src, S):
            return src.rearrange("p (s n) -> p s n", s=1).broadcast_to([128, S, _NB])

        # ---- L0 features: fa0 = A-tents; m0 = fa0 - 4*B  (fused 6*M3)
        nc.vector._custom_dve(
            TENT3, out=fa0[:, :].rearrange("p (s n) -> p s n", s=4),
            in0=bcast(u16, 4), s0=jc2[:, :], s1=2.0, imm2=2.0)
        off = 0
        for npg in _L0_SPLIT:
            sl = slice(off * _NB, (off + npg) * _NB)
            nc.vector._custom_dve(
                M3SUB, out=m0[:, sl].rearrange("p (s n) -> p s n", s=npg),
                in0=bcast(u16, npg),
                in1=fa0[:, sl].rearrange("p (s n) -> p s n", s=npg),
                s0=(jc2[:, :] if off == 0 else jc2b[:, :]), s1=2.0)
            off += npg

        # ---- L0 silu row
        nc.scalar.activation(s0t[0:64, :], u16[0:64, :], AF.Silu,
                             bias=e00b[0:64, :], scale=float(h0))

        # ---- L0 matmuls: silu chunk then 4 fused-M3 chunks
        nc.tensor.matmul(ph[:, :], lhsT=w0sb[:, 4 * 128:5 * 128], rhs=s0t[:, :],
                         start=True, stop=False)
        for k in range(4):
            nc.tensor.matmul(ph[:, :], lhsT=w0sb[:, k * 128:(k + 1) * 128],
                             rhs=m0[:, k * _NB:(k + 1) * _NB],
                             start=False, stop=(k == 3))

        # ---- u1' = (h + bias0 - e0_1)/h1 - 2  (fp16)
        nc.scalar.activation(u1t[:, :], ph[:, :], AF.Identity,
                             bias=prm[:, 0:1], scale=float(1.0 / h1))

        # ---- L1 silu row: silu(h1*u1 + e0_1) = silu(h + bias0)
        nc.scalar.activation(s1t[:, :], u1t[:, :], AF.Silu,
                             bias=sb1[:, :], scale=float(h1))

        # ---- L1 features
        nc.vector._custom_dve(
            TENT3, out=fa1[:, :].rearrange("p (s n) -> p s n", s=8),
            in0=bcast(u1t, 8), s0=2.0, s1=1.0, imm2=2.0)
        off = 0
        for npg in _L1_SPLIT:
            sl = slice(off * _NB, (off + npg) * _NB)
            nc.vector._custom_dve(
                M3SUB, out=m1[:, sl].rearrange("p (s n) -> p s n", s=npg),
                in0=bcast(u1t, npg),
                in1=fa1[:, sl].rearrange("p (s n) -> p s n", s=npg),
                s0=float(2.0 + off), s1=1.0)
            off += npg

        # ---- L1 matmuls: silu chunk then 8 fused-M3 chunks
        nc.tensor.matmul(po[:, :], lhsT=w1sb[:, 8 * 64:9 * 64], rhs=s1t[:, :],
                         start=True, stop=False)
        for s in range(8):
            nc.tensor.matmul(po[:, :], lhsT=w1sb[:, s * 64:(s + 1) * 64],
                             rhs=m1[:, s * _NB:(s + 1) * _NB],
                             start=False, stop=(s == 7))

        # ---- out^T = psum + bias1 ; store
        nc.scalar.activation(outT[:, :], po[:, :], AF.Identity,
                             bias=prm[0:64, 1:2], scale=1.0)
        nc.sync.dma_start(out=out_d, in_=outT[:, :])

    nc.compile()
    return nc


def _make_in_maps(inputs, e0_0, h0, e0_1, h1):
    x = np.asarray(inputs["x"], np.float32)
    u_full = ((x - np.float32(e0_0)) / np.float32(h0)).astype(np.float16)  # (B, 64)

    w0 = _pack_layer0(np.asarray(inputs["coef0"], np.float32),
                      np.asarray(inputs["scale_sp0"], np.float32),
                      np.asarray(inputs["scale_base0"], np.float32),
                      np.asarray(inputs["mask0"], np.float32)).reshape(128, 5 * 128)
    w1 = _pack_layer1(np.asarray(inputs["coef1"], np.float32),
                      np.asarray(inputs["scale_sp1"], np.float32),
                      np.asarray(inputs["scale_base1"], np.float32),
                      np.asarray(inputs["mask1"], np.float32)).reshape(128, 9 * 64)
    bias0 = np.asarray(inputs["bias0"], np.float32)
    bias1 = np.asarray(inputs["bias1"], np.float32)
    prm = np.zeros((128, 2), np.float32)
    prm[:, 0] = (bias0 - np.float32(e0_1)) / np.float32(h1)
    prm[0:64, 1] = bias1

    in_maps = []
    for c in range(_NCORES):
        xu = np.ascontiguousarray(u_full[c * _NB:(c + 1) * _NB, :].T)  # (64, NB) fp16
        u16 = np.concatenate([xu, xu], axis=0)                          # (128, NB)
        in_maps.append({
            "u16": np.ascontiguousarray(u16),
            "w0": np.ascontiguousarray(w0),
            "w1": np.ascontiguousarray(w1),
            "prm": prm,
        })
    return in_maps


# ---------------------------------------------------------------- entrypoint
def kernel(**inputs):
    global LAST_RESULTS
    x = np.asarray(inputs["x"])
    ok = (
        x.shape == (_BATCH, _W[0])
        and np.asarray(inputs["coef0"]).shape == (_W[0] * _W[1], _NBASIS)
        and np.asarray(inputs["coef1"]).shape == (_W[1] * _W[2], _NBASIS)
        and _uniform_shared(inputs["grid0"])
        and _uniform_shared(inputs["grid1"])
    )
    if not ok:
        return _np_reference(inputs)

    e0_0, h0 = _grid_params(inputs["grid0"])
    e0_1, h1 = _grid_params(inputs["grid1"])

    from concourse.bass_utils import run_bass_kernel_spmd

    nc = _build_program(e0_0, h0, e0_1, h1)
    in_maps = _make_in_maps(inputs, e0_0, h0, e0_1, h1)
    trace = bool(int(os.environ.get("KAN_TRACE", "0")))
    try:
        res = run_bass_kernel_spmd(nc, in_maps, list(range(_NCORES)), trace=trace)
    except ModuleNotFoundError:
        res = run_bass_kernel_spmd(nc, in_maps, list(range(_NCORES)), trace=False)
    LAST_RESULTS = res

    out = np.empty((_BATCH, _W[2]), np.float32)
    for c in range(_NCORES):
        out[c * _NB:(c + 1) * _NB, :] = res.results[c]["out"].T
    return out


if __name__ == "__main__":
    rng = np.random.default_rng(0)
    demo = {
        "x": rng.standard_normal((_BATCH, _W[0])).astype(np.float32),
    }
    for l, size in ((0, _W[0] * _W[1]), (1, _W[1] * _W[2])):
        demo[f"grid{l}"] = np.broadcast_to(
            np.linspace(-1, 1, _G + 1, dtype=np.float32), (size, _G + 1)).copy()
        demo[f"coef{l}"] = (rng.standard_normal((size, _NBASIS)) * 0.1).astype(np.float32)
        demo[f"scale_base{l}"] = rng.standard_normal(size).astype(np.float32) * 0.1 + 0.125
        demo[f"scale_sp{l}"] = np.ones(size, np.float32)
        demo[f"mask{l}"] = np.ones(size, np.float32)
        demo[f"bias{l}"] = np.zeros((_W[1], _W[2])[l], np.float32)
    out = kernel(**demo)
    ref = _np_reference(demo)
    err = np.abs(out - ref).max() / np.abs(ref).max()
    print("demo rel err:", err)


# revision 10
# speedup vs baseline: 1.1984x; 1.0219x over previous
"""KAN (Kolmogorov-Arnold Network) forward kernel for Trainium2, 8 NeuronCores.

Network: WIDTH=[64,128,64], BATCH=2048, cubic B-splines (K=3) on a shared
uniform grid of G=5 intervals over [-1,1], plus a SiLU residual branch per
edge.

Math: on the shared uniform extended knot grid every edge's spline is
    spline_s(x) = sum_c coef[s,c] * M3(u - c),   u = (x - e0)/h
with M3 the cardinal cubic B-spline.  With a common center m = c + 2:
    6*M3(u - c) = A(u) - 4*B(u)
    A(u) = relu(2 - |u - m|)^3 ,  B(u) = relu(1 - |u - m|)^3
Two custom DVE ops produce the fused feature directly (offsets generated
in-op with a paged counter, so no offset tables are streamed):
    KAN_TENT3 :  fa[p,s,n]  = relu(C2 - |u - (C0 + s*C1)|)^3        (= A)
    KAN_M3SUB :  m[p,s,n]   = fa - (2b)^2*b,  b = relu(1 - |u - (C0+s*C1)|)
so m = A - 4B = 6*M3 and one matmul chunk per basis index contracts the
fused features (half the matmul chunks and weight bytes of an unfused A/B
split).  The SiLU residual is a native Activation-engine Silu per layer.

Sharding: data-parallel over batch (2048/8 = 256 per core); spline params
repacked on host and replicated to all 8 cores.  No collectives.
"""

import os
import numpy as np

# ---------------------------------------------------------------- constants
_BATCH = 2048
_W = [64, 128, 64]
_NCORES = 8
_NB = _BATCH // _NCORES          # batch per core = 256
_K = 3                           # spline order
_G = 5                           # grid intervals
_NBASIS = _G + _K                # 8 basis functions per edge

_N_WARM = int(os.environ.get("KAN_NWARM", "6"))   # PE warm-up matmuls
# L1 fused-feature tail split (pages per KAN_M3SUB call)
_L1_SPLIT = tuple(int(x) for x in os.environ.get("KAN_L1SPLIT", "4,3,1").split(","))
_L0_SPLIT = tuple(int(x) for x in os.environ.get("KAN_L0SPLIT", "3,1").split(","))

LAST_RESULTS = None              # BassKernelResults of the most recent run


# ------------------------------------------------------------ numpy fallback
def _np_reference(inputs):
    """Exact numpy port of the reference (used only if the structural
    assumptions about the inputs do not hold)."""
    def extend_grid(grid, k):
        h = (grid[:, -1:] - grid[:, :1]) / (grid.shape[1] - 1)
        for _ in range(k):
            grid = np.concatenate([grid[:, :1] - h, grid, grid[:, -1:] + h], axis=1)
        return grid

    def b_spline_basis(x, grid, k):
        g = grid[:, :, None]
        xe = x[:, None, :]
        val = ((xe >= g[:, :-1]) & (xe < g[:, 1:])).astype(x.dtype)
        for p in range(1, k + 1):
            left = (xe - g[:, :-(p + 1)]) / (g[:, p:-1] - g[:, :-(p + 1)])
            right = (g[:, p + 1:] - xe) / (g[:, p + 1:] - xe * 0 - g[:, 1:-p])
            val = left * val[:, :-1] + right * val[:, 1:]
        return val

    def silu(v):
        return v / (1.0 + np.exp(-v))

    def kan_layer(x, grid, coef, scale_base, scale_sp, mask, in_dim, out_dim):
        batch = x.shape[0]
        xe = np.broadcast_to(x[:, None, :], (batch, out_dim, in_dim)).reshape(batch, -1).T
        base = silu(xe)
        B = b_spline_basis(xe, extend_grid(grid, _K), _K)
        spline = np.einsum('sc,scb->sb', coef, B)
        y = mask[:, None] * (scale_base[:, None] * base + scale_sp[:, None] * spline)
        return y.T.reshape(batch, out_dim, in_dim).sum(axis=2)

    x = np.asarray(inputs["x"], np.float32)
    h = kan_layer(x, inputs["grid0"], inputs["coef0"], inputs["scale_base0"],
                  inputs["scale_sp0"], inputs["mask0"], _W[0], _W[1]) + inputs["bias0"][None, :]
    out = kan_layer(h.astype(np.float32), inputs["grid1"], inputs["coef1"], inputs["scale_base1"],
                    inputs["scale_sp1"], inputs["mask1"], _W[1], _W[2]) + inputs["bias1"][None, :]
    return out.astype(np.float32)


def _grid_params(grid):
    """(e0, h) of the extended uniform grid; replicates extend_grid fp32 steps."""
    g0 = np.asarray(grid, np.float32)[0]
    h = np.float32((g0[-1] - g0[0]) / _G)
    e0 = np.float32(g0[0])
    for _ in range(_K):
        e0 = np.float32(e0 - h)
    return float(e0), float(h)


def _uniform_shared(grid):
    g = np.asarray(grid, np.float32)
    if not np.allclose(g, g[0][None, :], atol=1e-6):
        return False
    g0 = g[0]
    d = np.diff(g0)
    return np.allclose(d, d[0], rtol=1e-4, atol=1e-6) and d[0] > 0


# ------------------------------------------------------- custom DVE op setup
def _register_ops():
    """Register KAN_TENT3 (A-family tent-cube) and KAN_M3SUB (fused
    m = A - 4*B) custom DVE ops.  Offsets are generated in-op by a paged
    counter: center(s) = C0 + s*C1 (C0 may be a per-partition scalar AP)."""
    import dataclasses
    from concourse import dve_ops
    from concourse.dve_spec import (Spec, Src0, Src1, C0, C1, C2, One, relu,
                                    sq, lower, _has_src1, PageIdx, Bin, AluOp)
    from concourse.dve_uop import DveOpSpec

    def reg(name, spec, subdim):
        for op in dve_ops.OPS:
            if op.name == name:
                return op
        probe = dataclasses.replace(
            dve_ops.DveOp(name, spec, subdim=subdim, uops_sha={}), uops_sha={})
        dve_ops.OPS.append(probe)
        row = dve_ops._CUSTOM_DVE_ROW_BASE + len(dve_ops.OPS) - 1
        dve_ops._SUB_OPCODE_FOR_NAME[name] = row
        dve_ops.CUSTOM_DVE_SPECS[name] = spec
        shas = {}
        for ver in ("v3", "v4"):
            try:
                shas[ver] = DveOpSpec(
                    name=name, opcode=row, uops=lower(spec, ver=ver),
                    rd1_en=_has_src1(spec)).sha(ver)
            except Exception:
                pass
        final = dataclasses.replace(probe, uops_sha=shas)
        dve_ops.OPS[dve_ops.OPS.index(probe)] = final
        return final

    pg = PageIdx(C0, C1)
    ad = Bin(AluOp.ABSOLUTE_DIFF, Src0, pg)

    ma = C2 - ad
    ra = relu(ma)

    def _ref_a(in0, in1, s0, s1, imm2):
        x = in0.astype(np.float32)
        S = x.shape[1]
        off = (np.float32(s0) if np.isscalar(s0) else s0.astype(np.float32)[:, None, None]) \
            + np.arange(S, dtype=np.float32)[None, :, None] * np.float32(s1)
        rr = np.maximum(np.float32(imm2) - np.abs(x - off), 0.0)
        return (rr ** 3).astype(np.float32)

    tent3 = reg("KAN_TENT3", Spec(body=sq(ra) * ra, reference=_ref_a), True)

    mb = One - ad
    rb = relu(mb)
    b2 = rb + rb

    def _ref_b(in0, in1, s0, s1, imm2):
        x = in0.astype(np.float32)
        S = x.shape[1]
        off = (np.float32(s0) if np.isscalar(s0) else s0.astype(np.float32)[:, None, None]) \
            + np.arange(S, dtype=np.float32)[None, :, None] * np.float32(s1)
        bb = np.maximum(1.0 - np.abs(x - off), 0.0)
        return (in1.astype(np.float32) - 4.0 * bb ** 3).astype(np.float32)

    m3sub = reg("KAN_M3SUB", Spec(body=Src1 - sq(b2) * rb, reference=_ref_b), True)
    return tent3, m3sub


# ------------------------------------------------------------- host repacking
def _pack_layer0(coef, scale_sp, scale_base, mask):
    """W0 pack: (128, 5, 128) fp16.  Feature rows p=(j,i): j=p//64, i=p%64.
    Chunks 0..3: fused 6*M3 features for basis c=2k+j -> weight C/6.
    Chunk 4: silu rows (i<64) + zero pad."""
    C = (mask * scale_sp)[:, None] * coef / 6.0                # (8192, 8)
    C = C.reshape(_W[1], _W[0], _NBASIS)                       # (o, i, c)
    WB = (mask * scale_base).reshape(_W[1], _W[0])             # (o, i)
    w = np.zeros((128, 5, 128), np.float32)
    for k in range(4):
        for j in range(2):
            w[j * 64:(j + 1) * 64, k, :] = C[:, :, 2 * k + j].T
    w[0:64, 4, :] = WB.T
    return w.astype(np.float16)


def _pack_layer1(coef, scale_sp, scale_base, mask):
    """W1 pack: (128, 9, 64) fp16.  Rows p = input channel.
    Chunks 0..7: fused 6*M3 features, chunk 8: silu."""
    C = (mask * scale_sp)[:, None] * coef / 6.0
    C = C.reshape(_W[2], _W[1], _NBASIS)                       # (o, i, c)
    WB = (mask * scale_base).reshape(_W[2], _W[1])
    w = np.zeros((128, 9, 64), np.float32)
    for s in range(_NBASIS):
        w[:, s, :] = C[:, :, s].T
    w[:, 8, :] = WB.T
    return w.astype(np.float16)


# ------------------------------------------------------------- bass program
def _build_program(e0_0, h0, e0_1, h1):
    import concourse.bacc as bacc
    import concourse.tile as tile
    import concourse.mybir as mybir

    TENT3, M3SUB = _register_ops()

    dt = mybir.dt
    AF = mybir.ActivationFunctionType

    nc = bacc.Bacc("TRN2", target_bir_lowering=False, debug=False,
                   num_devices=_NCORES)

    u16_d = nc.dram_tensor("u16", [128, _NB], dt.float16, kind="ExternalInput").ap()
    w0_d = nc.dram_tensor("w0", [128, 5 * 128], dt.float16, kind="ExternalInput").ap()
    w1_d = nc.dram_tensor("w1", [128, 9 * 64], dt.float16, kind="ExternalInput").ap()
    prm_d = nc.dram_tensor("prm", [128, 2], dt.float32, kind="ExternalInput").ap()
    out_d = nc.dram_tensor("out", [64, _NB], dt.float32, kind="ExternalOutput").ap()

    import contextlib
    with tile.TileContext(nc) as tc, contextlib.ExitStack() as _ctx:
        pool = _ctx.enter_context(tc.tile_pool(name="main", bufs=1))
        ppool = _ctx.enter_context(tc.tile_pool(name="ps", bufs=1, space="PSUM"))

        u16 = pool.tile([128, _NB], dt.float16, name="u16", tag="u16")
        w0sb = pool.tile([128, 5 * 128], dt.float16, name="w0sb", tag="w0sb")
        w1sb = pool.tile([128, 9 * 64], dt.float16, name="w1sb", tag="w1sb")
        prm = pool.tile([128, 2], dt.float32, name="prm", tag="prm")
        jc2 = pool.tile([128, 1], dt.float32, name="jc2", tag="jc2")
        jc2b = pool.tile([128, 1], dt.float32, name="jc2b", tag="jc2b")
        e00b = pool.tile([128, 1], dt.float32, name="e00b", tag="e00b")
        sb1 = pool.tile([128, 1], dt.float32, name="sb1", tag="sb1")
        wzf = pool.tile([128, 1], dt.float32, name="wzf", tag="wzf")
        wz = pool.tile([128, _NB], dt.float16, name="wz", tag="wz")
        fa0 = pool.tile([128, 4 * _NB], dt.float16, name="fa0", tag="fa0")
        m0 = pool.tile([128, 4 * _NB], dt.float16, name="m0", tag="m0")
        s0t = pool.tile([128, _NB], dt.float16, name="s0t", tag="s0t")
        u1t = pool.tile([128, _NB], dt.float16, name="u1t", tag="u1t")
        fa1 = pool.tile([128, 8 * _NB], dt.float16, name="fa1", tag="fa1")
        m1 = pool.tile([128, 8 * _NB], dt.float16, name="m1", tag="m1")
        s1t = pool.tile([128, _NB], dt.float16, name="s1t", tag="s1t")
        outT = pool.tile([64, _NB], dt.float32, name="outT", tag="outT")
        aw = pool.tile([64, 1], dt.float16, name="aw", tag="aw")
        ph = ppool.tile([128, _NB], dt.float32, name="ph", tag="ph")
        po = ppool.tile([64, _NB], dt.float32, name="po", tag="po")
        if _N_WARM:
            pw = ppool.tile([128, _NB], dt.float32, name="pw", tag="pw")

        # ---- input DMAs (SP queue; ordered by when each is first needed)
        nc.sync.dma_start(out=u16[:, :], in_=u16_d)
        nc.sync.dma_start(out=w0sb[:, :], in_=w0_d)
        nc.sync.dma_start(out=w1sb[:, :], in_=w1_d)
        nc.sync.dma_start(out=prm[:, :], in_=prm_d)

        # ---- Pool: constants (all tiny memsets, off the critical path)
        nc.gpsimd.memset(wzf[:, :], 0.0)
        nc.gpsimd.memset(wz[:, :], 0.0)
        nc.gpsimd.memset(jc2[0:64, :], 2.0)     # L0 tent centers: 2k + j + 2
        nc.gpsimd.memset(jc2[64:128, :], 3.0)
        nc.gpsimd.memset(jc2b[0:64, :], 8.0)    # L0 M3SUB tail piece (page 3)
        nc.gpsimd.memset(jc2b[64:128, :], 9.0)
        nc.gpsimd.memset(e00b[:, :], float(e0_0))
        nc.gpsimd.memset(sb1[:, :], float(e0_1))
        nc.gpsimd.memset(s0t[64:128, :], 0.0)

        # ---- Activation: preload the silu/identity table off the critical path
        if not int(os.environ.get("KAN_NOWARM_ACT", "0")):
            nc.scalar.activation(aw[:, :], wz[0:64, 0:1], AF.Silu,
                                 bias=wzf[0:64, :], scale=1.0)

        # ---- PE warm-up
        for _ in range(_N_WARM):
            nc.tensor.matmul(pw[:, :], lhsT=wz[:, 0:128], rhs=wz[:, :],
                             start=True, stop=True)

        def bcast(src, S):
            return src.rearrange("p (s n) -> p s n", s=1).broadcast_to([128, S, _NB])

        # ---- L0 features: fa0 = A-tents; m0 = fa0 - 4*B  (fused 6*M3)
        nc.vector._custom_dve(
            TENT3, out=fa0[:, :].rearrange("p (s n) -> p s n", s=4),
            in0=bcast(u16, 4), s0=jc2[:, :], s1=2.0, imm2=2.0)
        off = 0
        for npg in _L0_SPLIT:
            sl = slice(off * _NB, (off + npg) * _NB)
            nc.vector._custom_dve(
                M3SUB, out=m0[:, sl].rearrange("p (s n) -> p s n", s=npg),
                in0=bcast(u16, npg),
                in1=fa0[:, sl].rearrange("p (s n) -> p s n", s=npg),
                s0=(jc2[:, :] if off == 0 else jc2b[:, :]), s1=2.0)
            off += npg

        # ---- L0 silu row
        nc.scalar.activation(s0t[0:64, :], u16[0:64, :], AF.Silu,
                             bias=e00b[0:64, :], scale=float(h0))

        # ---- L0 matmuls: silu chunk then 4 fused-M3 chunks
        nc.tensor.matmul(ph[:, :], lhsT=w0sb[:, 4 * 128:5 * 128], rhs=s0t[:, :],
                         start=True, stop=False)
        for k in range(4):
            nc.tensor.matmul(ph[:, :], lhsT=w0sb[:, k * 128:(k + 1) * 128],
                             rhs=m0[:, k * _NB:(k + 1) * _NB],
                             start=False, stop=(k == 3))

        # ---- u1' = (h + bias0 - e0_1)/h1 - 2  (fp16)
        nc.scalar.activation(u1t[:, :], ph[:, :], AF.Identity,
                             bias=prm[:, 0:1], scale=float(1.0 / h1))

        # ---- L1 silu row: silu(h1*u1 + e0_1) = silu(h + bias0)
        nc.scalar.activation(s1t[:, :], u1t[:, :], AF.Silu,
                             bias=sb1[:, :], scale=float(h1))

        # ---- L1 features
        nc.vector._custom_dve(
            TENT3, out=fa1[:, :].rearrange("p (s n) -> p s n", s=8),
            in0=bcast(u1t, 8), s0=2.0, s1=1.0, imm2=2.0)
        off = 0
        for npg in _L1_SPLIT:
            sl = slice(off * _NB, (off + npg) * _NB)
            nc.vector._custom_dve(
                M3SUB, out=m1[:, sl].rearrange("p (s n) -> p s n", s=npg),
                in0=bcast(u1t, npg),
                in1=fa1[:, sl].rearrange("p (s n) -> p s n", s=npg),
                s0=float(2.0 + off), s1=1.0)
            off += npg

        # ---- L1 matmuls: silu chunk then 8 fused-M3 chunks
        nc.tensor.matmul(po[:, :], lhsT=w1sb[:, 8 * 64:9 * 64], rhs=s1t[:, :],
                         start=True, stop=False)
        for s in range(8):
            nc.tensor.matmul(po[:, :], lhsT=w1sb[:, s * 64:(s + 1) * 64],
                             rhs=m1[:, s * _NB:(s + 1) * _NB],
                             start=False, stop=(s == 7))

        # ---- out^T = psum + bias1 ; store
        nc.scalar.activation(outT[:, :], po[:, :], AF.Identity,
                             bias=prm[0:64, 1:2], scale=1.0)
        nc.sync.dma_start(out=out_d, in_=outT[:, :])

    nc.compile()
    return nc


def _make_in_maps(inputs, e0_0, h0, e0_1, h1):
    x = np.asarray(inputs["x"], np.float32)
    u_full = ((x - np.float32(e0_0)) / np.float32(h0)).astype(np.float16)  # (B, 64)

    w0 = _pack_layer0(np.asarray(inputs["coef0"], np.float32),
                      np.asarray(inputs["scale_sp0"], np.float32),
                      np.asarray(inputs["scale_base0"], np.float32),
                      np.asarray(inputs["mask0"], np.float32)).reshape(128, 5 * 128)
    w1 = _pack_layer1(np.asarray(inputs["coef1"], np.float32),
                      np.asarray(inputs["scale_sp1"], np.float32),
                      np.asarray(inputs["scale_base1"], np.float32),
                      np.asarray(inputs["mask1"], np.float32)).reshape(128, 9 * 64)
    bias0 = np.asarray(inputs["bias0"], np.float32)
    bias1 = np.asarray(inputs["bias1"], np.float32)
    prm = np.zeros((128, 2), np.float32)
    prm[:, 0] = (bias0 - np.float32(e0_1)) / np.float32(h1)
    prm[0:64, 1] = bias1

    in_maps = []
    for c in range(_NCORES):
        xu = np.ascontiguousarray(u_full[c * _NB:(c + 1) * _NB, :].T)  # (64, NB) fp16
        u16 = np.concatenate([xu, xu], axis=0)                          # (128, NB)
        in_maps.append({
            "u16": np.ascontiguousarray(u16),
            "w0": np.ascontiguousarray(w0),
            "w1": np.ascontiguousarray(w1),
            "prm": prm,
        })
    return in_maps


# ---------------------------------------------------------------- entrypoint
def kernel(**inputs):
    global LAST_RESULTS
    x = np.asarray(inputs["x"])
    ok = (
        x.shape == (_BATCH, _W[0])
        and np.asarray(inputs["coef0"]).shape == (_W[0] * _W[1], _NBASIS)
        and np.asarray(inputs["coef1"]).shape == (_W[1] * _W[2], _NBASIS)
        and _uniform_shared(inputs["grid0"])
        and _uniform_shared(inputs["grid1"])
    )
    if not ok:
        return _np_reference(inputs)

    e0_0, h0 = _grid_params(inputs["grid0"])
    e0_1, h1 = _grid_params(inputs["grid1"])

    from concourse.bass_utils import run_bass_kernel_spmd

    nc = _build_program(e0_0, h0, e0_1, h1)
    in_maps = _make_in_maps(inputs, e0_0, h0, e0_1, h1)
    trace = bool(int(os.environ.get("KAN_TRACE", "0")))
    try:
        res = run_bass_kernel_spmd(nc, in_maps, list(range(_NCORES)), trace=trace)
    except ModuleNotFoundError:
        res = run_bass_kernel_spmd(nc, in_maps, list(range(_NCORES)), trace=False)
    LAST_RESULTS = res

    out = np.empty((_BATCH, _W[2]), np.float32)
    for c in range(_NCORES):
        out[c * _NB:(c + 1) * _NB, :] = res.results[c]["out"].T
    return out


if __name__ == "__main__":
    rng = np.random.default_rng(0)
    demo = {
        "x": rng.standard_normal((_BATCH, _W[0])).astype(np.float32),
    }
    for l, size in ((0, _W[0] * _W[1]), (1, _W[1] * _W[2])):
        demo[f"grid{l}"] = np.broadcast_to(
            np.linspace(-1, 1, _G + 1, dtype=np.float32), (size, _G + 1)).copy()
        demo[f"coef{l}"] = (rng.standard_normal((size, _NBASIS)) * 0.1).astype(np.float32)
        demo[f"scale_base{l}"] = rng.standard_normal(size).astype(np.float32) * 0.1 + 0.125
        demo[f"scale_sp{l}"] = np.ones(size, np.float32)
        demo[f"mask{l}"] = np.ones(size, np.float32)
        demo[f"bias{l}"] = np.zeros((_W[1], _W[2])[l], np.float32)
    out = kernel(**demo)
    ref = _np_reference(demo)
    err = np.abs(out - ref).max() / np.abs(ref).max()
    print("demo rel err:", err)
